# revision 2
# baseline (speedup 1.0000x reference)
"""Trainium2 Bass kernel: dwconv-QKV attention block, data-parallel over batch on 8 cores."""
import sys

sys.path.insert(0, "/opt/trn_rl_repo")

import numpy as np
import ml_dtypes

from concourse import bass, bacc, mybir, tile
from concourse.bass_utils import run_bass_kernel_spmd
from concourse.masks import make_identity

F32 = mybir.dt.float32
BF16 = mybir.dt.bfloat16
NCORES = 8
B, T, C, HEADS = 16, 1025, 768, 12
BL = B // NCORES  # batches per core
EPS = 1e-5
SCALE = float(C) ** -0.5
HW_ = 76  # per-head strip in v'/o: 64 v-cols + 12 one-hot denom cols


def _build(nc):
    x = nc.declare_dram_parameter("x", [BL, T, C], F32, isOutput=False)
    diag = nc.declare_dram_parameter("diag", [3, 6, 9, 128, 128], BF16, isOutput=False)
    biasp = nc.declare_dram_parameter("biasp", [3, 6, 128], F32, isOutput=False)
    pwt = nc.declare_dram_parameter("pwt", [12, 64, C], BF16, isOutput=False)
    oneh = nc.declare_dram_parameter("oneh", [128, 144], BF16, isOutput=False)
    selp = nc.declare_dram_parameter("selp", [12, C], BF16, isOutput=False)
    out = nc.declare_dram_parameter("out", [BL, T, C], F32, isOutput=True)

    from contextlib import ExitStack
    with nc.allow_low_precision(reason="bf16 compute, rel-err budget 2e-2"), \
         tile.TileContext(nc, trace_sim=False) as tc, ExitStack() as stk:
        sing = stk.enter_context(tc.tile_pool(name="sing", bufs=1))
        ident = sing.tile([128, 128], BF16, tag="ident")
        make_identity(nc, ident[:])
        onehs = sing.tile([128, 144], BF16, tag="onehs")
        nc.sync.dma_start(onehs[:], oneh[:])
        selps = sing.tile([HW_, C], BF16, tag="selps")
        nc.sync.dma_start(selps[64:HW_, :], selp[:])

        qT = [[sing.tile([128, T], BF16, tag=f"qT{b}_{cc}", name=f"qT{b}_{cc}") for cc in range(6)] for b in range(BL)]
        kT = [[sing.tile([128, T], BF16, tag=f"kT{b}_{cc}", name=f"kT{b}_{cc}") for cc in range(6)] for b in range(BL)]
        vp = [[sing.tile([128, 12 * HW_], BF16, tag=f"vp{b}_{tk}", name=f"vp{b}_{tk}") for tk in range(9)] for b in range(BL)]
        oT = [[sing.tile([64, T], BF16, tag=f"oT{b}_{h}", name=f"oT{b}_{h}") for h in range(12)] for b in range(BL)]
        dd = [sing.tile([HW_, T], F32, tag=f"dd{b}", name=f"dd{b}") for b in range(BL)]
        rr = [sing.tile([HW_, T], BF16, tag=f"rr{b}", name=f"rr{b}") for b in range(BL)]
        pw_sb = [sing.tile([64, C], BF16, tag=f"pw{h}", name=f"pw{h}") for h in range(12)]
        for h in range(12):
            nc.sync.dma_start(pw_sb[h][:], pwt[h])

        # ---------------- phase 1+2: transpose x, conv, build qT/kT/vp -------------
        with tc.tile_pool(name="cwt", bufs=1) as cwt, \
             tc.tile_pool(name="ld", bufs=2) as ld, \
             tc.tile_pool(name="xtp", bufs=6) as xtp, \
             tc.tile_pool(name="tp_ps", bufs=4, space="PSUM") as tp_ps, \
             tc.tile_pool(name="cv_ps", bufs=1, space="PSUM") as cv_ps:
            bsb = [[cwt.tile([128, 1], F32, tag=f"b{q}_{cc}", name=f"b{q}_{cc}") for cc in range(6)] for q in range(3)]
            for q in range(3):
                for cc in range(6):
                    nc.sync.dma_start(bsb[q][cc][:], biasp[q, cc].rearrange("(a b) -> a b", b=1))

            for b in range(BL):
                clsf = ld.tile([1, C], F32, tag="clsf")
                nc.sync.dma_start(clsf[:], x[b, 0:1, :])
                clsb = ld.tile([1, C], BF16, tag="clsb")
                nc.any.tensor_copy(clsb[:], clsf[:])
                clscf = ld.tile([128, 6], F32, tag="clscf")
                nc.sync.dma_start(clscf[:], x[b, 0, :].rearrange("(cc p) -> p cc", p=128))
                clscb = ld.tile([128, 6], BF16, tag="clscb")
                nc.any.tensor_copy(clscb[:], clscf[:])

                xT = [xtp.tile([128, 1160], BF16, tag="xtp", name=f"xT{i}") for i in range(6)]
                for cc in range(6):
                    nc.vector.memset(xT[cc][:], 0.0)
                    nc.any.tensor_copy(qT[b][cc][:, 0:1], clscb[:, cc:cc + 1])
                    nc.any.tensor_copy(kT[b][cc][:, 0:1], clscb[:, cc:cc + 1])
                for tk in range(9):
                    # one-hot denominator columns (64..75 of each head strip)
                    dst = vp[b][tk][:].rearrange("p (h s) -> p h s", s=HW_)[:, :, 64:HW_]
                    nc.any.tensor_copy(dst, onehs[:].rearrange("p (h s) -> p h s", s=12))
                for h in range(12):
                    nc.any.tensor_copy(
                        vp[b][0][0:1, h * HW_:h * HW_ + 64], clsb[:, h * 64:(h + 1) * 64])

                for tt in range(8):
                    xn = ld.tile([128, C], F32, tag="xn")
                    nc.sync.dma_start(xn[:], x[b, 1 + tt * 128:1 + (tt + 1) * 128, :])
                    xb = ld.tile([128, C], BF16, tag="xb")
                    nc.any.tensor_copy(xb[:], xn[:])
                    for cc in range(6):
                        pt = tp_ps.tile([128, 128], BF16, tag="tp")
                        nc.tensor.transpose(pt[:], xb[:, cc * 128:(cc + 1) * 128], ident[:])
                        dst = xT[cc][:, 0:1156].rearrange("p (r w) -> p r w", w=34)[
                            :, 1 + tt * 4:1 + tt * 4 + 4, 1:33]
                        nc.any.tensor_copy(dst, pt[:].rearrange("p (r w) -> p r w", w=32))

                for q in range(3):
                    for cc in range(6):
                        # dense conv over padded coords [35,1123): out col i = pos 35+i
                        cp = cv_ps.tile([128, 1536], F32, tag="cv")
                        dg = ld.tile([128, 1152], BF16, tag="dg")
                        nc.sync.dma_start(
                            dg[:].rearrange("p (t m) -> p t m", t=9),
                            diag[q, cc].rearrange("a b c -> b a c"))
                        for t in range(9):
                            dy, dx = t // 3, t % 3
                            for (o0, o1) in [(0, 512), (512, 1024), (1024, 1088)]:
                                off = 35 + o0 + (dy - 1) * 34 + (dx - 1)
                                nc.tensor.matmul(
                                    cp[:, o0:o1], dg[:, t * 128:(t + 1) * 128],
                                    xT[cc][:, off:off + (o1 - o0)],
                                    start=(t == 0), stop=(t == 8))
                        cps = cp[:, 0:1088].rearrange("p (r w) -> p r w", w=34)[:, :, 0:32]
                        if q < 2:
                            dstT = (qT if q == 0 else kT)[b][cc]
                            nc.any.tensor_scalar_add(
                                dstT[:, 1:T].rearrange("p (r w) -> p r w", w=32),
                                cps, bsb[q][cc][:])
                        else:
                            vt = ld.tile([128, 1024], BF16, tag="vt")
                            nc.any.tensor_scalar_add(
                                vt[:].rearrange("p (r w) -> p r w", w=32),
                                cps, bsb[2][cc][:])
                            for g in range(1, 9):
                                pt = tp_ps.tile([128, 128], BF16, tag="tp")
                                nc.tensor.transpose(
                                    pt[:], vt[:, (g - 1) * 128:g * 128], ident[:])
                                dst = vp[b][g][:, 2 * cc * HW_:(2 * cc + 2) * HW_].rearrange(
                                    "p (h s) -> p h s", s=HW_)[:, :, 0:64]
                                nc.any.tensor_copy(
                                    dst, pt[:].rearrange("p (h d) -> p h d", d=64))

        # ------------- phase 3: attention, both batches interleaved -----------------
        for b in range(BL):
            nc.vector.memset(dd[b][64:HW_, :], 0.0)
        with tc.tile_pool(name="scp", bufs=2, space="PSUM") as scp, \
             tc.tile_pool(name="opp", bufs=1, space="PSUM") as opp, \
             tc.tile_pool(name="tlp", bufs=1, space="PSUM") as tlp, \
             tc.tile_pool(name="eap", bufs=3) as eap:
            sctl = tlp.tile([128, 216], F32, tag="sctl")
            otl = tlp.tile([HW_, 24], F32, tag="otl")
            for h in range(12):
                for b in range(BL):
                    hb = h * 2 + b
                    cc, r0 = h // 2, (h % 2) * 64
                    ops = opp.tile([HW_, 1024], F32, tag="o")
                    for g in range(9):
                        tsz = 1 if g == 0 else 128
                        c0 = 0 if g == 0 else 1 + (g - 1) * 128
                        sc = scp.tile([128, 1024], F32, tag="sc")
                        for (l0, l1) in [(0, 512), (512, 1024)]:
                            nc.tensor.matmul(
                                sc[0:tsz, l0:l1],
                                kT[b][cc][r0:r0 + 64, c0:c0 + tsz],
                                qT[b][cc][r0:r0 + 64, l0:l1], start=True, stop=True)
                        nc.tensor.matmul(
                            sctl[0:tsz, hb * 9 + g:hb * 9 + g + 1],
                            kT[b][cc][r0:r0 + 64, c0:c0 + tsz],
                            qT[b][cc][r0:r0 + 64, 1024:1025], start=True, stop=True,
                            skip_group_check=True)
                        E = eap.tile([128, T], BF16, tag="E")
                        nc.scalar.activation(
                            E[0:tsz, 0:1024], sc[0:tsz, :],
                            mybir.ActivationFunctionType.Exp, scale=SCALE)
                        nc.scalar.activation(
                            E[0:tsz, 1024:1025], sctl[0:tsz, hb * 9 + g:hb * 9 + g + 1],
                            mybir.ActivationFunctionType.Exp, scale=SCALE)
                        for (l0, l1) in [(0, 512), (512, 1024)]:
                            nc.tensor.matmul(
                                ops[:, l0:l1], vp[b][g][0:tsz, h * HW_:(h + 1) * HW_],
                                E[0:tsz, l0:l1], start=(g == 0), stop=(g == 8))
                        nc.tensor.matmul(
                            otl[:, hb:hb + 1], vp[b][g][0:tsz, h * HW_:(h + 1) * HW_],
                            E[0:tsz, 1024:1025], start=(g == 0), stop=(g == 8),
                            skip_group_check=True)
                    nc.any.tensor_copy(oT[b][h][:, 0:1024], ops[0:64, :])
                    nc.any.tensor_copy(oT[b][h][:, 1024:1025], otl[0:64, hb:hb + 1])
                    nc.vector.tensor_add(dd[b][64:HW_, 0:1024], dd[b][64:HW_, 0:1024],
                                         ops[64:HW_, :])
                    nc.vector.tensor_add(dd[b][64:HW_, 1024:1025],
                                         dd[b][64:HW_, 1024:1025], otl[64:HW_, hb:hb + 1])

        for b in range(BL):
            nc.vector.reciprocal(rr[b][64:HW_, :], dd[b][64:HW_, :])
            with tc.tile_pool(name=f"rb{b}", bufs=2, space="PSUM") as rbp, \
                 tc.tile_pool(name=f"rs{b}", bufs=2) as rsp:
                for h in range(12):
                    rp = rbp.tile([64, T], F32, tag="rp")
                    for (l0, l1) in [(0, 512), (512, 1024), (1024, 1025)]:
                        nc.tensor.matmul(rp[:, l0:l1],
                                         selps[64:HW_, h * 64:(h + 1) * 64],
                                         rr[b][64:HW_, l0:l1], start=True, stop=True)
                    rbs = rsp.tile([64, T], BF16, tag="rbs")
                    nc.any.tensor_copy(rbs[:], rp[:])
                    nc.vector.tensor_mul(oT[b][h][:], oT[b][h][:], rbs[:])

            with tc.tile_pool(name=f"pr{b}", bufs=2, space="PSUM") as prp, \
                 tc.tile_pool(name=f"po{b}", bufs=3) as pop:
                for lt in range(9):
                    lsz = 128 if lt < 8 else 1
                    pp = prp.tile([128, C], F32, tag="pr")
                    for h in range(12):
                        for (e0, e1) in [(0, 512), (512, 768)]:
                            nc.tensor.matmul(
                                pp[0:lsz, e0:e1], oT[b][h][:, lt * 128:lt * 128 + lsz],
                                pw_sb[h][:, e0:e1], start=(h == 0), stop=(h == 11))
                    ob = pop.tile([128, C], F32, tag="po")
                    nc.any.tensor_copy(ob[0:lsz, :], pp[0:lsz, :])
                    nc.sync.dma_start(out[b, lt * 128:lt * 128 + lsz, :], ob[0:lsz, :])
    return nc


_CACHE = {}


def _get_nc():
    if "nc" not in _CACHE:
        nc = bacc.Bacc("TRN2", target_bir_lowering=False, debug=False,
                       enable_asserts=False, num_devices=NCORES)
        _build(nc)
        nc.compile()
        _CACHE["nc"] = nc
    return _CACHE["nc"]


def _prep_weights(w, g, bb, m, v):
    s = (np.asarray(g) / np.sqrt(np.asarray(v) + EPS)).astype(np.float32)
    w9 = np.asarray(w).reshape(C, 9).astype(np.float32) * s[:, None]
    bias = (np.asarray(bb) - np.asarray(m) * s).astype(np.float32)
    return w9, bias


def _make_in_maps(inputs):
    x = np.asarray(inputs["x"], dtype=np.float32)
    diag = np.zeros((3, 6, 9, 128, 128), dtype=ml_dtypes.bfloat16)
    biasp = np.zeros((3, 6, 128), dtype=np.float32)
    idx = np.arange(128)
    for q, pre in enumerate(["q", "k", "v"]):
        w9, bias = _prep_weights(
            inputs[f"w_{pre}"], inputs[f"bn_{pre}_g"], inputs[f"bn_{pre}_b"],
            inputs[f"bn_{pre}_m"], inputs[f"bn_{pre}_v"])
        for cc in range(6):
            for t in range(9):
                diag[q, cc, t, idx, idx] = w9[cc * 128:(cc + 1) * 128, t].astype(
                    ml_dtypes.bfloat16)
            biasp[q, cc] = bias[cc * 128:(cc + 1) * 128]
    pwt = np.ascontiguousarray(
        np.asarray(inputs["proj_w"], np.float32).T.reshape(12, 64, C)).astype(
        ml_dtypes.bfloat16)
    oneh = np.tile(np.eye(12).reshape(1, 144), (128, 1)).astype(ml_dtypes.bfloat16)
    selp = np.kron(np.eye(12, dtype=np.float32),
                   np.ones((1, 64), np.float32)).astype(ml_dtypes.bfloat16)
    in_maps = []
    for ci in range(NCORES):
        in_maps.append({
            "x": np.ascontiguousarray(x[ci * BL:(ci + 1) * BL]),
            "diag": diag, "biasp": biasp, "pwt": pwt,
            "oneh": np.ascontiguousarray(oneh), "selp": np.ascontiguousarray(selp),
        })
    return in_maps


def kernel(x, w_q, bn_q_g, bn_q_b, bn_q_m, bn_q_v,
           w_k, bn_k_g, bn_k_b, bn_k_m, bn_k_v,
           w_v, bn_v_g, bn_v_b, bn_v_m, bn_v_v,
           proj_w, proj_b, h, w, **_):
    inputs = dict(x=x, w_q=w_q, bn_q_g=bn_q_g, bn_q_b=bn_q_b, bn_q_m=bn_q_m,
                  bn_q_v=bn_q_v, w_k=w_k, bn_k_g=bn_k_g, bn_k_b=bn_k_b,
                  bn_k_m=bn_k_m, bn_k_v=bn_k_v, w_v=w_v, bn_v_g=bn_v_g,
                  bn_v_b=bn_v_b, bn_v_m=bn_v_m, bn_v_v=bn_v_v, proj_w=proj_w)
    nc = _get_nc()
    in_maps = _make_in_maps(inputs)
    res = run_bass_kernel_spmd(nc, in_maps, core_ids=list(range(NCORES)))
    _CACHE["res"] = res
    outs = [res.results[ci]["out"] for ci in range(NCORES)]
    full = np.concatenate(outs, axis=0).astype(np.float32)
    full += np.asarray(proj_b, np.float32)[None, None, :]
    return full

